# revision 1
# baseline (speedup 1.0000x reference)
"""Expert-parallel MoE kernel for Trainium2 (8 NeuronCores).

Strategy (expert-parallel, per sharding hint):
  - Host: sort the T*top_k dispatch pairs by expert, scale each dispatched
    token by its gate score (gate folds into the linear map's input), pad
    each expert's token group to a fixed capacity CAP, lay out as [K, M]
    (pre-transposed for the PE's lhsT operand), cast to bf16.
  - Device (SPMD, core m owns experts 2m and 2m+1): Z_e = X_e^T.T @ W_e
    as tiled bf16 matmuls with fp32 PSUM accumulation.
  - Host: scatter Z rows back to dispatch pairs, sum top_k contributions,
    add the (gate-weighted) expert biases.
"""

import numpy as np
import ml_dtypes

NUM_EXPERT = 16
D = 1024
TOP_K = 2
T = 2048
N_CORES = 8
EPC = NUM_EXPERT // N_CORES  # experts per core
CAP = 384                    # per-expert dispatch capacity (multiple of 128)
KT = D // 128                # contraction tiles
NT = D // 512                # output free-dim tiles (one PSUM bank each)
MT = CAP // 128              # token tiles

TRACE = False                # set by test harness to collect an NTFF profile
LAST_RESULT = None           # BassKernelResults of the most recent run

_NC = None


def _build_nc():
    from concourse import bacc, tile
    import concourse.mybir as mybir

    bf16 = mybir.dt.bfloat16
    f32 = mybir.dt.float32

    nc = bacc.Bacc("TRN2", target_bir_lowering=False, debug=False,
                   num_devices=N_CORES)
    w = nc.declare_dram_parameter("w", [EPC, KT, 128, D], bf16, isOutput=False)
    xt = nc.declare_dram_parameter("xt", [EPC, KT, 128, CAP], bf16,
                                   isOutput=False)
    z = nc.declare_dram_parameter("z", [EPC, CAP, D], f32, isOutput=True)

    with tile.TileContext(nc, num_cores=N_CORES) as tc:
        with (
            tc.tile_pool(name="wp", bufs=1) as wp,
            tc.tile_pool(name="xp", bufs=1) as xp,
            tc.tile_pool(name="pp", bufs=4, space="PSUM") as pp,
            tc.tile_pool(name="op", bufs=4) as op,
        ):
            wts, xts = {}, {}
            for e in range(EPC):
                for k in range(KT):
                    wt = wp.tile([128, D], bf16, tag=f"w{e}_{k}")
                    nc.sync.dma_start(wt[:], w[e, k])
                    wts[e, k] = wt
                    xtt = xp.tile([128, CAP], bf16, tag=f"x{e}_{k}")
                    nc.sync.dma_start(xtt[:], xt[e, k])
                    xts[e, k] = xtt

            for e in range(EPC):
                for m in range(MT):
                    for n in range(NT):
                        ps = pp.tile([128, 512], f32)
                        for k in range(KT):
                            nc.tensor.matmul(
                                ps[:],
                                xts[e, k][:, m * 128:(m + 1) * 128],
                                wts[e, k][:, n * 512:(n + 1) * 512],
                                start=(k == 0),
                                stop=(k == KT - 1),
                            )
                        ot = op.tile([128, 512], f32)
                        nc.any.tensor_copy(ot[:], ps[:])
                        nc.sync.dma_start(
                            z[e, m * 128:(m + 1) * 128,
                              n * 512:(n + 1) * 512], ot[:])
    nc.compile()
    return nc


def kernel(inp, gate_idx, gate_score, W, b):
    global _NC, LAST_RESULT
    from concourse.bass_utils import run_bass_kernel_spmd

    inp = np.ascontiguousarray(np.asarray(inp, dtype=np.float32))
    gi = np.asarray(gate_idx).astype(np.int64)
    gs = np.asarray(gate_score, dtype=np.float32)
    W = np.asarray(W, dtype=np.float32)
    b = np.asarray(b, dtype=np.float32)

    P = T * TOP_K
    fe = gi.reshape(P)
    fg = gs.reshape(P)
    tok = np.arange(P) // TOP_K

    order = np.argsort(fe, kind="stable")
    counts = np.bincount(fe, minlength=NUM_EXPERT)
    starts = np.zeros(NUM_EXPERT + 1, np.int64)
    np.cumsum(counts, out=starts[1:])
    rank = np.arange(P) - starts[fe[order]]
    ok = rank < CAP
    sel = order[ok]
    rnk = rank[ok]

    xpad = np.zeros((NUM_EXPERT, CAP, D), np.float32)
    xpad[fe[sel], rnk] = inp[tok[sel]] * fg[sel, None]
    xt_dev = np.ascontiguousarray(
        xpad.reshape(NUM_EXPERT, CAP, KT, 128).transpose(0, 2, 3, 1)
    ).astype(ml_dtypes.bfloat16)
    w_dev = np.ascontiguousarray(
        W.reshape(NUM_EXPERT, KT, 128, D)).astype(ml_dtypes.bfloat16)

    if _NC is None:
        _NC = _build_nc()

    in_maps = [
        {"w": w_dev[c * EPC:(c + 1) * EPC],
         "xt": xt_dev[c * EPC:(c + 1) * EPC]}
        for c in range(N_CORES)
    ]
    res = run_bass_kernel_spmd(_NC, in_maps, list(range(N_CORES)),
                               trace=TRACE)
    LAST_RESULT = res
    zall = np.concatenate([r["z"] for r in res.results], axis=0)  # [E,CAP,D]

    zpairs = np.zeros((P, D), np.float32)
    zpairs[sel] = zall[fe[sel], rnk]
    overflow = order[~ok]
    for p in overflow:  # essentially never taken (CAP >> expected max count)
        zpairs[p] = (inp[tok[p]] * fg[p]) @ W[fe[p]]

    y = zpairs.reshape(T, TOP_K, D).sum(axis=1)
    y += (gs[:, :, None] * b[gi]).sum(axis=1)
    return y.astype(np.float32)


# revision 5
# speedup vs baseline: 1.0759x; 1.0759x over previous
"""Expert-parallel MoE kernel for Trainium2 (8 NeuronCores).

Strategy (expert-parallel, per sharding hint):
  - Host: sort the T*top_k dispatch pairs by expert, scale each dispatched
    token by its gate score (gate folds into the linear map's input), pad
    each expert's token group to a fixed capacity CAP, lay out as [K, M]
    (pre-transposed for the PE's lhsT operand), cast to bf16.
  - Device (SPMD, core m owns experts 2m and 2m+1): Z_e = X_e^T.T @ W_e
    as tiled bf16 matmuls with fp32 PSUM accumulation.
  - Host: scatter Z rows back to dispatch pairs, sum top_k contributions,
    add the (gate-weighted) expert biases.
"""

import numpy as np
import ml_dtypes

NUM_EXPERT = 16
D = 1024
TOP_K = 2
T = 2048
N_CORES = 8
EPC = NUM_EXPERT // N_CORES  # experts per core
CAP = 384                    # per-expert dispatch capacity (multiple of 128)
KT = D // 128                # contraction tiles
NT = D // 512                # output free-dim tiles (one PSUM bank each)
MT = CAP // 128              # token tiles

TRACE = False                # set by test harness to collect an NTFF profile
LAST_RESULT = None           # BassKernelResults of the most recent run

_NC = None


def _build_nc():
    from concourse import bacc, tile
    import concourse.mybir as mybir

    bf16 = mybir.dt.bfloat16
    f32 = mybir.dt.float32

    nc = bacc.Bacc("TRN2", target_bir_lowering=False, debug=False,
                   num_devices=N_CORES)
    w = nc.declare_dram_parameter("w", [EPC, KT, 128, D], bf16, isOutput=False)
    xt = nc.declare_dram_parameter("xt", [EPC, KT, 128, CAP], bf16,
                                   isOutput=False)
    z = nc.declare_dram_parameter("z", [EPC, CAP, D], bf16, isOutput=True)

    with tile.TileContext(nc, num_cores=N_CORES) as tc:
        with (
            tc.tile_pool(name="wp", bufs=1) as wp,
            tc.tile_pool(name="xp", bufs=1) as xp,
            tc.tile_pool(name="pp", bufs=4, space="PSUM") as pp,
            tc.tile_pool(name="op", bufs=4) as op,
        ):
            # interleave x/w loads per k so the PE can stream k-by-k
            # right behind the DMA; all loads on the sync HWDGE ring,
            # all stores on the scalar HWDGE ring.
            wts, xts = {}, {}
            for e in range(EPC):
                for k in range(KT):
                    xtt = xp.tile([128, CAP], bf16, tag=f"x{e}_{k}")
                    nc.sync.dma_start(xtt[:], xt[e, k])
                    xts[e, k] = xtt
                    wt = wp.tile([128, D], bf16, tag=f"w{e}_{k}")
                    nc.sync.dma_start(wt[:], w[e, k])
                    wts[e, k] = wt

            for e in range(EPC):
                for m in range(MT):
                    # pair the two N halves per stationary lhsT load
                    ps = [pp.tile([128, 512], f32, name=f"ps{n}",
                                  tag=f"ps{n}")
                          for n in range(NT)]
                    for k in range(KT):
                        for n in range(NT):
                            nc.tensor.matmul(
                                ps[n][:],
                                xts[e, k][:, m * 128:(m + 1) * 128],
                                wts[e, k][:, n * 512:(n + 1) * 512],
                                start=(k == 0),
                                stop=(k == KT - 1),
                            )
                    ot = op.tile([128, D], bf16)
                    for n in range(NT):
                        nc.any.tensor_copy(
                            ot[:, n * 512:(n + 1) * 512], ps[n][:])
                    nc.scalar.dma_start(
                        z[e, m * 128:(m + 1) * 128, :], ot[:])
    nc.compile()
    return nc


def kernel(inp, gate_idx, gate_score, W, b):
    global _NC, LAST_RESULT
    from concourse.bass_utils import run_bass_kernel_spmd

    inp = np.ascontiguousarray(np.asarray(inp, dtype=np.float32))
    gi = np.asarray(gate_idx).astype(np.int64)
    gs = np.asarray(gate_score, dtype=np.float32)
    W = np.asarray(W, dtype=np.float32)
    b = np.asarray(b, dtype=np.float32)

    P = T * TOP_K
    fe = gi.reshape(P)
    fg = gs.reshape(P)
    tok = np.arange(P) // TOP_K

    order = np.argsort(fe, kind="stable")
    counts = np.bincount(fe, minlength=NUM_EXPERT)
    starts = np.zeros(NUM_EXPERT + 1, np.int64)
    np.cumsum(counts, out=starts[1:])
    rank = np.arange(P) - starts[fe[order]]
    ok = rank < CAP
    sel = order[ok]
    rnk = rank[ok]

    xpad = np.zeros((NUM_EXPERT, CAP, D), np.float32)
    xpad[fe[sel], rnk] = inp[tok[sel]] * fg[sel, None]
    xt_dev = np.ascontiguousarray(
        xpad.reshape(NUM_EXPERT, CAP, KT, 128).transpose(0, 2, 3, 1)
    ).astype(ml_dtypes.bfloat16)
    w_dev = np.ascontiguousarray(
        W.reshape(NUM_EXPERT, KT, 128, D)).astype(ml_dtypes.bfloat16)

    if _NC is None:
        _NC = _build_nc()

    in_maps = [
        {"w": w_dev[c * EPC:(c + 1) * EPC],
         "xt": xt_dev[c * EPC:(c + 1) * EPC]}
        for c in range(N_CORES)
    ]
    res = run_bass_kernel_spmd(_NC, in_maps, list(range(N_CORES)),
                               trace=TRACE)
    LAST_RESULT = res
    zall = np.concatenate(
        [np.asarray(r["z"]).astype(np.float32) for r in res.results],
        axis=0)  # [E,CAP,D]

    zpairs = np.zeros((P, D), np.float32)
    zpairs[sel] = zall[fe[sel], rnk]
    overflow = order[~ok]
    for p in overflow:  # essentially never taken (CAP >> expected max count)
        zpairs[p] = (inp[tok[p]] * fg[p]) @ W[fe[p]]

    y = zpairs.reshape(T, TOP_K, D).sum(axis=1)
    y += (gs[:, :, None] * b[gi]).sum(axis=1)
    return y.astype(np.float32)


# revision 7
# speedup vs baseline: 1.1146x; 1.0360x over previous
"""Expert-parallel MoE kernel for Trainium2 (8 NeuronCores).

Strategy (expert-parallel, per sharding hint):
  - Host: sort the T*top_k dispatch pairs by expert, scale each dispatched
    token by its gate score (gate folds into the linear map's input), pad
    each expert's token group to a fixed capacity CAP, lay out as [K, M]
    (pre-transposed for the PE's lhsT operand), cast to bf16.
  - Device (SPMD, core m owns experts 2m and 2m+1): Z_e = X_e^T.T @ W_e
    as tiled bf16 matmuls with fp32 PSUM accumulation.
  - Host: scatter Z rows back to dispatch pairs, sum top_k contributions,
    add the (gate-weighted) expert biases.
"""

import numpy as np
import ml_dtypes

NUM_EXPERT = 16
D = 1024
TOP_K = 2
T = 2048
N_CORES = 8
EPC = NUM_EXPERT // N_CORES  # experts per core
CAP = 384                    # per-expert dispatch capacity (multiple of 128)
KT = D // 128                # contraction tiles
NT = D // 512                # output free-dim tiles (one PSUM bank each)
MT = CAP // 128              # token tiles

TRACE = False                # set by test harness to collect an NTFF profile
LAST_RESULT = None           # BassKernelResults of the most recent run

_NC = None


def _build_nc():
    from concourse import bacc, tile
    import concourse.mybir as mybir

    bf16 = mybir.dt.bfloat16
    f32 = mybir.dt.float32

    nc = bacc.Bacc("TRN2", target_bir_lowering=False, debug=False,
                   num_devices=N_CORES)
    # p-major DRAM layouts: each SBUF partition's data is one contiguous
    # run in DRAM, so DMA descriptors are multi-KB instead of sub-KB.
    w = nc.declare_dram_parameter("w", [EPC, 128, KT, D], bf16,
                                  isOutput=False)
    xt = nc.declare_dram_parameter("xt", [EPC, 128, KT, CAP], bf16,
                                   isOutput=False)
    z = nc.declare_dram_parameter("z", [EPC, CAP, D], bf16, isOutput=True)

    KC = 2            # k-tiles per DMA chunk
    NCH = KT // KC    # chunks per expert

    with tile.TileContext(nc, num_cores=N_CORES) as tc:
        with (
            tc.tile_pool(name="wp", bufs=1) as wp,
            tc.tile_pool(name="xp", bufs=1) as xp,
            tc.tile_pool(name="pp", bufs=1, space="PSUM") as pp,
            tc.tile_pool(name="op", bufs=4) as op,
        ):
            # loads on the sync HWDGE ring (x chunk then w chunk per k
            # block, expert-major), stores on the scalar HWDGE ring
            wts, xts = {}, {}
            for e in range(EPC):
                for c in range(NCH):
                    xtt = xp.tile([128, KC, CAP], bf16, tag=f"x{e}_{c}")
                    nc.sync.dma_start(
                        xtt[:], xt[e][:, c * KC:(c + 1) * KC, :])
                    xts[e, c] = xtt
                    wt = wp.tile([128, KC, D], bf16, tag=f"w{e}_{c}")
                    nc.sync.dma_start(
                        wt[:], w[e][:, c * KC:(c + 1) * KC, :])
                    wts[e, c] = wt

            # k-outer: all 6 (m, n) PSUM groups of an expert accumulate
            # in parallel, so the PE consumes each k chunk as it lands
            for e in range(EPC):
                pss = {}
                for m in range(MT):
                    for n in range(NT):
                        pss[m, n] = pp.tile([128, 512], f32,
                                            name=f"ps{m}{n}",
                                            tag=f"ps{m}{n}")
                for k in range(KT):
                    c, kb = divmod(k, KC)
                    for m in range(MT):
                        for n in range(NT):
                            nc.tensor.matmul(
                                pss[m, n][:],
                                xts[e, c][:, kb, m * 128:(m + 1) * 128],
                                wts[e, c][:, kb, n * 512:(n + 1) * 512],
                                start=(k == 0),
                                stop=(k == KT - 1),
                            )
                for m in range(MT):
                    ot = op.tile([128, D], bf16)
                    for n in range(NT):
                        nc.any.tensor_copy(
                            ot[:, n * 512:(n + 1) * 512], pss[m, n][:])
                    nc.scalar.dma_start(
                        z[e, m * 128:(m + 1) * 128, :], ot[:])
    nc.compile()
    return nc


def kernel(inp, gate_idx, gate_score, W, b):
    global _NC, LAST_RESULT
    from concourse.bass_utils import run_bass_kernel_spmd

    inp = np.ascontiguousarray(np.asarray(inp, dtype=np.float32))
    gi = np.asarray(gate_idx).astype(np.int64)
    gs = np.asarray(gate_score, dtype=np.float32)
    W = np.asarray(W, dtype=np.float32)
    b = np.asarray(b, dtype=np.float32)

    P = T * TOP_K
    fe = gi.reshape(P)
    fg = gs.reshape(P)
    tok = np.arange(P) // TOP_K

    order = np.argsort(fe, kind="stable")
    counts = np.bincount(fe, minlength=NUM_EXPERT)
    starts = np.zeros(NUM_EXPERT + 1, np.int64)
    np.cumsum(counts, out=starts[1:])
    rank = np.arange(P) - starts[fe[order]]
    ok = rank < CAP
    sel = order[ok]
    rnk = rank[ok]

    xpad = np.zeros((NUM_EXPERT, CAP, D), np.float32)
    xpad[fe[sel], rnk] = inp[tok[sel]] * fg[sel, None]
    # p-major device layouts: [E, p, k, ...] with per-partition data
    # contiguous in DRAM (big DMA descriptors)
    xt_dev = np.ascontiguousarray(
        xpad.reshape(NUM_EXPERT, CAP, KT, 128).transpose(0, 3, 2, 1)
    ).astype(ml_dtypes.bfloat16)
    w_dev = np.ascontiguousarray(
        W.reshape(NUM_EXPERT, KT, 128, D).transpose(0, 2, 1, 3)
    ).astype(ml_dtypes.bfloat16)

    if _NC is None:
        _NC = _build_nc()

    in_maps = [
        {"w": w_dev[c * EPC:(c + 1) * EPC],
         "xt": xt_dev[c * EPC:(c + 1) * EPC]}
        for c in range(N_CORES)
    ]
    res = run_bass_kernel_spmd(_NC, in_maps, list(range(N_CORES)),
                               trace=TRACE)
    LAST_RESULT = res
    zall = np.concatenate(
        [np.asarray(r["z"]).astype(np.float32) for r in res.results],
        axis=0)  # [E,CAP,D]

    zpairs = np.zeros((P, D), np.float32)
    zpairs[sel] = zall[fe[sel], rnk]
    overflow = order[~ok]
    for p in overflow:  # essentially never taken (CAP >> expected max count)
        zpairs[p] = (inp[tok[p]] * fg[p]) @ W[fe[p]]

    y = zpairs.reshape(T, TOP_K, D).sum(axis=1)
    y += (gs[:, :, None] * b[gi]).sum(axis=1)
    return y.astype(np.float32)


# revision 10
# speedup vs baseline: 1.3051x; 1.1709x over previous
"""Expert-parallel MoE kernel for Trainium2 (8 NeuronCores).

Strategy (expert-parallel, per sharding hint):
  - Host: sort the T*top_k dispatch pairs by expert, scale each dispatched
    token by its gate score (gate folds into the linear map's input), pad
    each expert's token group to a fixed capacity CAP, lay out as [K, M]
    (pre-transposed for the PE's lhsT operand), cast to bf16.
  - Device (SPMD, core m owns experts 2m and 2m+1): Z_e = X_e^T.T @ W_e
    as tiled bf16 matmuls with fp32 PSUM accumulation.
  - Host: scatter Z rows back to dispatch pairs, sum top_k contributions,
    add the (gate-weighted) expert biases.
"""

import numpy as np
import ml_dtypes

NUM_EXPERT = 16
D = 1024
TOP_K = 2
T = 2048
N_CORES = 8
EPC = NUM_EXPERT // N_CORES  # experts per core
CAP = 256                    # per-expert dispatch capacity (multiple of 128)
KT = D // 128                # contraction tiles
NT = D // 512                # output free-dim tiles (one PSUM bank each)
MT = CAP // 128              # token tiles

TRACE = False                # set by test harness to collect an NTFF profile
LAST_RESULT = None           # BassKernelResults of the most recent run

_NC = None


def _build_nc():
    from concourse import bacc, tile
    import concourse.mybir as mybir

    bf16 = mybir.dt.bfloat16
    f32 = mybir.dt.float32

    nc = bacc.Bacc("TRN2", target_bir_lowering=False, debug=False,
                   num_devices=N_CORES)
    # p-major DRAM layouts: each SBUF partition's data is one contiguous
    # run in DRAM, so DMA descriptors are multi-KB instead of sub-KB.
    w = nc.declare_dram_parameter("w", [EPC, 128, KT, D], bf16,
                                  isOutput=False)
    xt = nc.declare_dram_parameter("xt", [EPC, 128, KT, CAP], bf16,
                                   isOutput=False)
    z = nc.declare_dram_parameter("z", [EPC, CAP, D], bf16, isOutput=True)

    KC = 2            # k-tiles per W DMA chunk
    NCH = KT // KC    # W chunks per expert

    with tile.TileContext(nc, num_cores=N_CORES) as tc:
        with (
            tc.tile_pool(name="wp", bufs=1) as wp,
            tc.tile_pool(name="xp", bufs=1) as xp,
            tc.tile_pool(name="pp", bufs=2, space="PSUM") as pp,
            tc.tile_pool(name="op", bufs=4) as op,
        ):
            # loads on the sync HWDGE ring (whole-expert x, then W in k
            # chunks); stores split across the scalar and sync rings
            wts, xts = {}, {}
            for e in range(EPC):
                xtt = xp.tile([128, KT, CAP], bf16, tag=f"x{e}")
                nc.sync.dma_start(xtt[:], xt[e])
                xts[e] = xtt
                for c in range(NCH):
                    wt = wp.tile([128, KC, D], bf16, tag=f"w{e}_{c}")
                    nc.sync.dma_start(
                        wt[:], w[e][:, c * KC:(c + 1) * KC, :])
                    wts[e, c] = wt

            # k-outer: all 4 (m, n) PSUM groups of an expert accumulate
            # in parallel, so the PE consumes each k chunk as it lands;
            # 4 banks/expert * bufs=2 = 8 banks -> experts double-buffer
            for e in range(EPC):
                pss = {}
                for m in range(MT):
                    for n in range(NT):
                        pss[m, n] = pp.tile([128, 512], f32,
                                            name=f"ps{m}{n}",
                                            tag=f"ps{m}{n}")
                for k in range(KT):
                    c, kb = divmod(k, KC)
                    for m in range(MT):
                        for n in range(NT):
                            nc.tensor.matmul(
                                pss[m, n][:],
                                xts[e][:, k, m * 128:(m + 1) * 128],
                                wts[e, c][:, kb, n * 512:(n + 1) * 512],
                                start=(k == 0),
                                stop=(k == KT - 1),
                            )
                for m in range(MT):
                    ot = op.tile([128, D], bf16)
                    for n in range(NT):
                        nc.any.tensor_copy(
                            ot[:, n * 512:(n + 1) * 512], pss[m, n][:])
                    eng = nc.scalar if e == 0 else nc.sync
                    eng.dma_start(
                        z[e, m * 128:(m + 1) * 128, :], ot[:])
    nc.compile()
    return nc


def kernel(inp, gate_idx, gate_score, W, b):
    global _NC, LAST_RESULT
    from concourse.bass_utils import run_bass_kernel_spmd

    inp = np.ascontiguousarray(np.asarray(inp, dtype=np.float32))
    gi = np.asarray(gate_idx).astype(np.int64)
    gs = np.asarray(gate_score, dtype=np.float32)
    W = np.asarray(W, dtype=np.float32)
    b = np.asarray(b, dtype=np.float32)

    P = T * TOP_K
    fe = gi.reshape(P)
    fg = gs.reshape(P)
    tok = np.arange(P) // TOP_K

    order = np.argsort(fe, kind="stable")
    counts = np.bincount(fe, minlength=NUM_EXPERT)
    starts = np.zeros(NUM_EXPERT + 1, np.int64)
    np.cumsum(counts, out=starts[1:])
    rank = np.arange(P) - starts[fe[order]]
    ok = rank < CAP
    sel = order[ok]
    rnk = rank[ok]

    xpad = np.zeros((NUM_EXPERT, CAP, D), np.float32)
    xpad[fe[sel], rnk] = inp[tok[sel]] * fg[sel, None]
    # p-major device layouts: [E, p, k, ...] with per-partition data
    # contiguous in DRAM (big DMA descriptors)
    xt_dev = np.ascontiguousarray(
        xpad.reshape(NUM_EXPERT, CAP, KT, 128).transpose(0, 3, 2, 1)
    ).astype(ml_dtypes.bfloat16)
    w_dev = np.ascontiguousarray(
        W.reshape(NUM_EXPERT, KT, 128, D).transpose(0, 2, 1, 3)
    ).astype(ml_dtypes.bfloat16)

    if _NC is None:
        _NC = _build_nc()

    in_maps = [
        {"w": w_dev[c * EPC:(c + 1) * EPC],
         "xt": xt_dev[c * EPC:(c + 1) * EPC]}
        for c in range(N_CORES)
    ]
    res = run_bass_kernel_spmd(_NC, in_maps, list(range(N_CORES)),
                               trace=TRACE)
    LAST_RESULT = res
    zall = np.concatenate(
        [np.asarray(r["z"]).astype(np.float32) for r in res.results],
        axis=0)  # [E,CAP,D]

    zpairs = np.zeros((P, D), np.float32)
    zpairs[sel] = zall[fe[sel], rnk]
    # exact f32 fallback for over-capacity pairs (~2% of dispatches)
    overflow = order[~ok]
    if overflow.size:
        fe_o = fe[overflow]
        for e in np.unique(fe_o):
            pi = overflow[fe_o == e]
            zpairs[pi] = (inp[tok[pi]] * fg[pi, None]) @ W[e]

    y = zpairs.reshape(T, TOP_K, D).sum(axis=1)
    y += (gs[:, :, None] * b[gi]).sum(axis=1)
    return y.astype(np.float32)


# revision 11
# speedup vs baseline: 1.3685x; 1.0486x over previous
"""Expert-parallel MoE kernel for Trainium2 (8 NeuronCores).

Strategy (expert-parallel, per sharding hint):
  - Host: sort the T*top_k dispatch pairs by expert, scale each dispatched
    token by its gate score (gate folds into the linear map's input), pad
    each expert's token group to a fixed capacity CAP, lay out as [K, M]
    (pre-transposed for the PE's lhsT operand), cast to bf16.
  - Device (SPMD, core m owns experts 2m and 2m+1): Z_e = X_e^T.T @ W_e
    as tiled bf16 matmuls with fp32 PSUM accumulation.
  - Host: scatter Z rows back to dispatch pairs, sum top_k contributions,
    add the (gate-weighted) expert biases.
"""

import numpy as np
import ml_dtypes

NUM_EXPERT = 16
D = 1024
TOP_K = 2
T = 2048
N_CORES = 8
EPC = NUM_EXPERT // N_CORES  # experts per core
CAP = 256                    # per-expert dispatch capacity (multiple of 128)
KT = D // 128                # contraction tiles
NT = D // 512                # output free-dim tiles (one PSUM bank each)
MT = CAP // 128              # token tiles

TRACE = False                # set by test harness to collect an NTFF profile
LAST_RESULT = None           # BassKernelResults of the most recent run

_NC = None


def _build_nc():
    from concourse import bacc, tile
    import concourse.mybir as mybir

    bf16 = mybir.dt.bfloat16
    f32 = mybir.dt.float32

    nc = bacc.Bacc("TRN2", target_bir_lowering=False, debug=False,
                   num_devices=N_CORES)
    # p-major DRAM layouts: each SBUF partition's data is one contiguous
    # run in DRAM, so DMA descriptors are multi-KB instead of sub-KB.
    w = nc.declare_dram_parameter("w", [EPC, 128, KT, D], bf16,
                                  isOutput=False)
    xt = nc.declare_dram_parameter("xt", [EPC, 128, KT, CAP], bf16,
                                   isOutput=False)
    z = nc.declare_dram_parameter("z", [EPC, CAP, D], bf16, isOutput=True)

    with tile.TileContext(nc, num_cores=N_CORES) as tc:
        with (
            tc.tile_pool(name="wp", bufs=1) as wp,
            tc.tile_pool(name="xp", bufs=1) as xp,
            tc.tile_pool(name="pp", bufs=2, space="PSUM") as pp,
            tc.tile_pool(name="op", bufs=4) as op,
        ):
            # per-k W chunks, x in half-expert chunks; stripe the loads
            # across both HWDGE rings (sync + scalar) in k order so the
            # PE can start early and never starves
            wts, xts = {}, {}
            rings = [nc.sync, nc.scalar]
            ri = 0
            for e in range(EPC):
                for h in range(2):
                    xtt = xp.tile([128, KT // 2, CAP], bf16,
                                  name=f"x{e}_{h}", tag=f"x{e}_{h}")
                    rings[ri % 2].dma_start(
                        xtt[:], xt[e][:, h * (KT // 2):(h + 1) * (KT // 2), :])
                    ri += 1
                    xts[e, h] = xtt
                    for k in range(h * (KT // 2), (h + 1) * (KT // 2)):
                        wt = wp.tile([128, D], bf16, name=f"w{e}_{k}",
                                     tag=f"w{e}_{k}")
                        rings[ri % 2].dma_start(wt[:], w[e][:, k, :])
                        ri += 1
                        wts[e, k] = wt

            # k-outer: all 4 (m, n) PSUM groups of an expert accumulate
            # in parallel, so the PE consumes each k chunk as it lands;
            # 4 banks/expert * bufs=2 = 8 banks -> experts double-buffer
            for e in range(EPC):
                pss = {}
                for m in range(MT):
                    for n in range(NT):
                        pss[m, n] = pp.tile([128, 512], f32,
                                            name=f"ps{m}{n}",
                                            tag=f"ps{m}{n}")
                for k in range(KT):
                    for m in range(MT):
                        for n in range(NT):
                            nc.tensor.matmul(
                                pss[m, n][:],
                                xts[e, k // (KT // 2)][
                                    :, k % (KT // 2),
                                    m * 128:(m + 1) * 128],
                                wts[e, k][:, n * 512:(n + 1) * 512],
                                start=(k == 0),
                                stop=(k == KT - 1),
                            )
                for m in range(MT):
                    ot = op.tile([128, D], bf16)
                    for n in range(NT):
                        nc.any.tensor_copy(
                            ot[:, n * 512:(n + 1) * 512], pss[m, n][:])
                    eng = rings[(e * MT + m) % 2]
                    eng.dma_start(
                        z[e, m * 128:(m + 1) * 128, :], ot[:])
    nc.compile()
    return nc


def kernel(inp, gate_idx, gate_score, W, b):
    global _NC, LAST_RESULT
    from concourse.bass_utils import run_bass_kernel_spmd

    inp = np.ascontiguousarray(np.asarray(inp, dtype=np.float32))
    gi = np.asarray(gate_idx).astype(np.int64)
    gs = np.asarray(gate_score, dtype=np.float32)
    W = np.asarray(W, dtype=np.float32)
    b = np.asarray(b, dtype=np.float32)

    P = T * TOP_K
    fe = gi.reshape(P)
    fg = gs.reshape(P)
    tok = np.arange(P) // TOP_K

    order = np.argsort(fe, kind="stable")
    counts = np.bincount(fe, minlength=NUM_EXPERT)
    starts = np.zeros(NUM_EXPERT + 1, np.int64)
    np.cumsum(counts, out=starts[1:])
    rank = np.arange(P) - starts[fe[order]]
    ok = rank < CAP
    sel = order[ok]
    rnk = rank[ok]

    xpad = np.zeros((NUM_EXPERT, CAP, D), np.float32)
    xpad[fe[sel], rnk] = inp[tok[sel]] * fg[sel, None]
    # p-major device layouts: [E, p, k, ...] with per-partition data
    # contiguous in DRAM (big DMA descriptors)
    xt_dev = np.ascontiguousarray(
        xpad.reshape(NUM_EXPERT, CAP, KT, 128).transpose(0, 3, 2, 1)
    ).astype(ml_dtypes.bfloat16)
    w_dev = np.ascontiguousarray(
        W.reshape(NUM_EXPERT, KT, 128, D).transpose(0, 2, 1, 3)
    ).astype(ml_dtypes.bfloat16)

    if _NC is None:
        _NC = _build_nc()

    in_maps = [
        {"w": w_dev[c * EPC:(c + 1) * EPC],
         "xt": xt_dev[c * EPC:(c + 1) * EPC]}
        for c in range(N_CORES)
    ]
    res = run_bass_kernel_spmd(_NC, in_maps, list(range(N_CORES)),
                               trace=TRACE)
    LAST_RESULT = res
    zall = np.concatenate(
        [np.asarray(r["z"]).astype(np.float32) for r in res.results],
        axis=0)  # [E,CAP,D]

    zpairs = np.zeros((P, D), np.float32)
    zpairs[sel] = zall[fe[sel], rnk]
    # exact f32 fallback for over-capacity pairs (~2% of dispatches)
    overflow = order[~ok]
    if overflow.size:
        fe_o = fe[overflow]
        for e in np.unique(fe_o):
            pi = overflow[fe_o == e]
            zpairs[pi] = (inp[tok[pi]] * fg[pi, None]) @ W[e]

    y = zpairs.reshape(T, TOP_K, D).sum(axis=1)
    y += (gs[:, :, None] * b[gi]).sum(axis=1)
    return y.astype(np.float32)
